# revision 1
# baseline (speedup 1.0000x reference)
"""InternVisionAttention TRN2 kernel: 8-core tensor-parallel over heads.

Layout strategy (per core c, heads 2c..2c+1):
  - qkv column-parallel: qT/kT computed transposed [feat(128) x S], v natural.
  - RMS-norm over full embed dim needs a cross-core sumsq AllReduce (16KB).
  - rope applied on transposed layout via partition-shifted DVE ops.
  - attention per cu_seqlens segment only (block-diagonal -> no masking).
    scoresT layout [s_k x s_q]; exp on ACT with per-partition k-norm scale;
    softmax denominator comes free from a ones-column appended to v.
  - AllToAll redistributes attention output so each core projects its own
    S/8 slice with the full proj matrix (row-parallel proj, no reduce).
"""
import math
import numpy as np

import bass_rust
import concourse.bass as bass
import concourse.mybir as mybir
import concourse.tile as tile
from concourse.bass_utils import run_bass_kernel_spmd
from concourse.vector_clock import ScopedClock

F32 = mybir.dt.float32
AF = mybir.ActivationFunctionType
N_CORES = 8
S, E, H, D = 2048, 1024, 16, 64
HPC = H // N_CORES          # heads per core = 2
FPC = HPC * D               # features per core = 128
SLC = S // N_CORES          # sequence slice per core = 256
EPS = 1e-6

# ---- walrus workaround: sync engine allows 1 sem wait per instruction ----
def _drain_and_barrier(self, tick_clock, wait_clock):
    nc = self.nc
    drain_inst = nc.sync.drain()
    wait_clock.add_sem_waits(drain_inst.ins,
                             ScopedClock({None: tick_clock.global_clock}))
    si = drain_inst.ins.sync_info
    if si is not None and len(si.on_wait) > 1:
        waits = list(si.on_wait)
        drain_inst.ins.sync_info = bass_rust.SyncInfo(
            on_wait=waits[:1], on_update=list(si.on_update))
        for i in range(1, len(waits)):
            nop = nc.sync.nop(nofuse=True)
            nop.ins.sync_info = bass_rust.SyncInfo(
                on_wait=waits[i:i + 1], on_update=[])
    nc.all_engine_barrier()
    assert self.sems is not None
    popped = nc._tile_sem_poison_stack.pop()
    assert popped is self._sem_poison
    nc.clear_and_free_semaphores(list(self.sems.allocated().values()))
    nc.all_engine_barrier()

tile.TileContext._drain_and_barrier = _drain_and_barrier


def _split_multiwaits(nc):
    """Walrus here allows only one sync wait per instruction: hoist extra
    waits onto same-engine nops inserted just before (in-order engines)."""
    n = 0
    for bb in nc.m.functions[0].blocks:
        insts = bb.instructions
        i = 0
        while i < len(insts):
            inst = insts[i]
            si = inst.sync_info
            if si is not None and len(si.on_wait) > 1:
                waits = list(si.on_wait)
                inst.sync_info = bass_rust.SyncInfo(
                    on_wait=waits[-1:], on_update=list(si.on_update))
                for w in waits[:-1]:
                    nop = mybir.InstNoOp(name=f"mwsplit_{n}",
                                         engine=inst.engine, bass_nofuse=True)
                    nop.sync_info = bass_rust.SyncInfo(on_wait=[w], on_update=[])
                    insts.insert(i, nop)
                    i += 1
                    n += 1
            i += 1


def _build(cu):
    """Build the Bass program, specialized on cu_seqlens values."""
    segs = [(int(cu[i]), int(cu[i + 1])) for i in range(len(cu) - 1)
            if int(cu[i + 1]) > int(cu[i])]

    nc = bass.Bass(num_devices=N_CORES)
    hT = nc.dram_tensor("hT", [E, S], F32, kind="ExternalInput")
    wqT = nc.dram_tensor("wqT", [E, FPC], F32, kind="ExternalInput")
    wkT = nc.dram_tensor("wkT", [E, FPC], F32, kind="ExternalInput")
    wvT = nc.dram_tensor("wvT", [E, FPC], F32, kind="ExternalInput")
    bq = nc.dram_tensor("bq", [FPC, 1], F32, kind="ExternalInput")
    bk = nc.dram_tensor("bk", [FPC, 1], F32, kind="ExternalInput")
    bv = nc.dram_tensor("bv", [1, FPC], F32, kind="ExternalInput")
    wqn = nc.dram_tensor("wqn", [FPC, 1], F32, kind="ExternalInput")
    wkn = nc.dram_tensor("wkn", [FPC, 1], F32, kind="ExternalInput")
    projT = nc.dram_tensor("projT", [E, E], F32, kind="ExternalInput")
    bo = nc.dram_tensor("bo", [1, E], F32, kind="ExternalInput")
    frT = nc.dram_tensor("frT", [D // 2, S], F32, kind="ExternalInput")
    out = nc.dram_tensor("out", [SLC, E], F32, kind="ExternalOutput")

    with tile.TileContext(nc) as tc:
        with tc.tile_pool(name="persist", bufs=1) as pp, \
             tc.tile_pool(name="dram", bufs=1, space="DRAM") as dram:
            # persistent tiles
            wq_s = pp.tile([128, 8, FPC], F32)
            wk_s = pp.tile([128, 8, FPC], F32)
            wv_s = pp.tile([128, 8, FPC], F32)
            nc.sync.dma_start(wq_s[:], wqT.ap().rearrange("(eo p) o -> p eo o", p=128))
            nc.sync.dma_start(wk_s[:], wkT.ap().rearrange("(eo p) o -> p eo o", p=128))
            nc.sync.dma_start(wv_s[:], wvT.ap().rearrange("(eo p) o -> p eo o", p=128))
            bq_s = pp.tile([FPC, 1], F32)
            bk_s = pp.tile([FPC, 1], F32)
            bv_s = pp.tile([1, FPC], F32)
            wqn_s = pp.tile([FPC, 1], F32)
            wkn_s = pp.tile([FPC, 1], F32)
            bo_s = pp.tile([1, E], F32)
            nc.sync.dma_start(bq_s[:], bq.ap())
            nc.sync.dma_start(bk_s[:], bk.ap())
            nc.sync.dma_start(bv_s[:], bv.ap())
            nc.sync.dma_start(wqn_s[:], wqn.ap())
            nc.sync.dma_start(wkn_s[:], wkn.ap())
            nc.sync.dma_start(bo_s[:], bo.ap())
            ones_r = pp.tile([1, 128], F32)      # ones row (K=1 lhsT tricks)
            ones_c = pp.tile([128, 1], F32)      # ones column (sumsq rhs)
            nc.vector.memset(ones_r[:], 1.0)
            nc.vector.memset(ones_c[:], 1.0)
            halfpi = pp.tile([128, 1], F32)
            nc.vector.memset(halfpi[:], math.pi / 2)
            epsq = pp.tile([1, 1], F32)
            nc.vector.memset(epsq[:], float(D) * EPS)
            epsk = pp.tile([128, 1], F32)
            nc.vector.memset(epsk[:], EPS)

            cosT = pp.tile([128, S], F32)
            sinT = pp.tile([128, S], F32)
            qT = pp.tile([128, S], F32)          # raw then roped/normed q
            kT = pp.tile([128, S], F32)
            v_s = pp.tile([128, 16, HPC, D + 1], F32)   # +ones column
            nc.vector.memset(v_s[:, :, :, D:D + 1], 1.0)
            outT = pp.tile([128, S], F32)
            sq_q = pp.tile([2, S], F32)          # row0: q sumsq, row1 unused
            ks_p = pp.tile([128, 16], F32)       # k sumsq partition-major
            fq = pp.tile([1, S], F32)
            fk = pp.tile([128, 16], F32)

            # ---------------- phase 1: qkv ----------------
            with tc.tile_pool(name="hpool", bufs=1) as hp, \
                 tc.tile_pool(name="p1ps", bufs=2, space="PSUM") as p1ps, \
                 tc.tile_pool(name="p1pv", bufs=2, space="PSUM") as p1pv, \
                 tc.tile_pool(name="p1sq", bufs=1, space="PSUM") as p1sq, \
                 tc.tile_pool(name="sqtmp", bufs=2) as sqt:
                h_s = hp.tile([128, 8, S], F32)
                nc.sync.dma_start(h_s[:], hT.ap().rearrange("(eo p) s -> p eo s", p=128))
                fr = hp.tile([128, S], F32)
                for b in range(4):
                    nc.sync.dma_start(fr[b * 32:(b + 1) * 32, :], frT.ap())
                nc.scalar.activation(sinT[:], fr[:], AF.Sin)
                nc.scalar.activation(cosT[:], fr[:], AF.Sin, bias=halfpi[:])

                for sc in range(4):
                    sl = slice(sc * 512, (sc + 1) * 512)
                    pq = p1ps.tile([128, 512], F32, tag="pqk")
                    pk = p1ps.tile([128, 512], F32, tag="pqk")
                    for eo in range(8):
                        nc.tensor.matmul(pq[:], wq_s[:, eo, :], h_s[:, eo, sl],
                                         start=(eo == 0), stop=(eo == 7))
                    for eo in range(8):
                        nc.tensor.matmul(pk[:], wk_s[:, eo, :], h_s[:, eo, sl],
                                         start=(eo == 0), stop=(eo == 7))
                    # bias (per-partition) evac
                    nc.scalar.activation(qT[:, sl], pq[:], AF.Identity, bias=bq_s[:])
                    nc.scalar.activation(kT[:, sl], pk[:], AF.Identity, bias=bk_s[:])
                    # sumsq partials
                    qsq = sqt.tile([128, 512], F32, tag="sq")
                    ksq = sqt.tile([128, 512], F32, tag="sq")
                    nc.scalar.activation(qsq[:], qT[:, sl], AF.Square)
                    nc.scalar.activation(ksq[:], kT[:, sl], AF.Square)
                    psq = p1sq.tile([1, 512], F32, tag="psq")
                    nc.tensor.matmul(psq[:], ones_c[:], qsq[:])
                    nc.scalar.activation(sq_q[0:1, sl], psq[:], AF.Identity)
                    for ss in range(4):
                        pks = p1sq.tile([128, 1], F32, tag="pks")
                        nc.tensor.matmul(pks[:], ksq[:, ss * 128:(ss + 1) * 128],
                                         ones_c[:])
                        nc.scalar.activation(
                            ks_p[:, sc * 4 + ss:sc * 4 + ss + 1], pks[:], AF.Identity)
                    # norm-weight mul (before rope)
                    nc.vector.tensor_scalar_mul(qT[:, sl], qT[:, sl], wqn_s[:])
                    nc.vector.tensor_scalar_mul(kT[:, sl], kT[:, sl], wkn_s[:])
                    # v natural with ones-trick bias
                    for ss in range(4):
                        so = sc * 4 + ss
                        pv = p1pv.tile([128, FPC], F32, tag="pv")
                        ssl = slice(so * 128, (so + 1) * 128)
                        for eo in range(8):
                            nc.tensor.matmul(pv[:], h_s[:, eo, ssl], wv_s[:, eo, :],
                                             start=(eo == 0), stop=False)
                        nc.tensor.matmul(pv[:], ones_r[:1, :], bv_s[:],
                                         start=False, stop=True)
                        for h in range(HPC):
                            nc.scalar.activation(v_s[:, so, h, 0:D],
                                                 pv[:, h * D:(h + 1) * D], AF.Identity)

                # cross-core sumsq AllReduce (packed into one buffer)
                cc_in = dram.tile([6144], F32)
                cc_out = dram.tile([6144], F32)
                nc.sync.dma_start(
                    cc_in[0:4096].rearrange("(a b) -> a b", a=2), sq_q[:])
                nc.sync.dma_start(
                    cc_in[4096:6144].rearrange("(a b) -> a b", a=128), ks_p[:])
                nc.gpsimd.collective_compute(
                    "AllReduce", mybir.AluOpType.add,
                    replica_groups=[list(range(N_CORES))],
                    ins=[cc_in.opt()], outs=[cc_out.opt()])
                nc.sync.dma_start(
                    sq_q[:], cc_out[0:4096].rearrange("(a b) -> a b", a=2))
                nc.sync.dma_start(
                    ks_p[:], cc_out[4096:6144].rearrange("(a b) -> a b", a=128))
                # fq = (1/8)*rsqrt(var+eps); fk = rsqrt(var+eps)
                nc.scalar.activation(fq[:], sq_q[0:1, :], AF.Sqrt,
                                     scale=float(D) / E, bias=epsq[:])
                nc.vector.reciprocal(fq[:], fq[:])
                nc.scalar.activation(fk[:], ks_p[:], AF.Sqrt,
                                     scale=1.0 / E, bias=epsk[:])
                nc.vector.reciprocal(fk[:], fk[:])

                # ---- rope (q,k) then q *= fq broadcast ----
                with tc.tile_pool(name="ropet", bufs=2) as rp, \
                     tc.tile_pool(name="bps", bufs=2, space="PSUM") as bps:
                    for t in (qT, kT):
                        tmp = rp.tile([128, S], F32, tag="ropetmp")
                        for h in range(HPC):
                            lo = h * D
                            mid = lo + D // 2
                            hi = lo + D
                            nc.vector.tensor_copy(tmp[lo:mid, :], t[mid:hi, :])
                            nc.vector.tensor_copy(tmp[mid:hi, :], t[lo:mid, :])
                        nc.vector.tensor_mul(tmp[:], tmp[:], sinT[:])
                        nc.vector.tensor_mul(t[:], t[:], cosT[:])
                        for h in range(HPC):
                            lo = h * D
                            mid = lo + D // 2
                            hi = lo + D
                            nc.vector.tensor_sub(t[lo:mid, :], t[lo:mid, :],
                                                 tmp[lo:mid, :])
                            nc.vector.tensor_add(t[mid:hi, :], t[mid:hi, :],
                                                 tmp[mid:hi, :])
                    for nqc in range(4):
                        sl = slice(nqc * 512, (nqc + 1) * 512)
                        pb = bps.tile([128, 512], F32, tag="pb")
                        nc.tensor.matmul(pb[:], ones_r[:1, :], fq[0:1, sl])
                        nc.vector.tensor_mul(qT[:, sl], qT[:, sl], pb[:])

            # ---------------- phase 2: attention ----------------
            with tc.tile_pool(name="projp", bufs=1) as prp, \
                 tc.tile_pool(name="expp", bufs=3) as ep, \
                 tc.tile_pool(name="recp", bufs=2) as rcp, \
                 tc.tile_pool(name="aps", bufs=3, space="PSUM") as aps, \
                 tc.tile_pool(name="apo", bufs=2, space="PSUM") as apo, \
                 tc.tile_pool(name="apb", bufs=2, space="PSUM") as apb:
                proj_s = prp.tile([128, 8, E], F32)
                nc.sync.dma_start(
                    proj_s[:], projT.ap().rearrange("(ko p) e -> p ko e", p=128))

                for h in range(HPC):
                    hsl = slice(h * D, (h + 1) * D)
                    for (s0, s1) in segs:
                        # k chunks on the 128 grid
                        kch = []
                        k0 = s0
                        while k0 < s1:
                            k1 = min(s1, (k0 // 128 + 1) * 128)
                            kch.append((k0, k1))
                            k0 = k1
                        q0 = s0
                        while q0 < s1:
                            q1 = min(s1, q0 + 512)
                            nq = q1 - q0
                            po = apo.tile([D + 1, 512], F32, tag="po")
                            for ki, (k0, k1) in enumerate(kch):
                                mk = k1 - k0
                                so, p0 = k0 // 128, k0 % 128
                                ps = aps.tile([128, 512], F32, tag="ps")
                                nc.tensor.matmul(ps[:mk, :nq], kT[hsl, k0:k1],
                                                 qT[hsl, q0:q1])
                                et = ep.tile([128, 512], F32, tag="et")
                                nc.scalar.activation(
                                    et[:mk, :nq], ps[:mk, :nq], AF.Exp,
                                    scale=fk[p0:p0 + mk, so:so + 1])
                                nc.tensor.matmul(
                                    po[:, :nq], v_s[p0:p0 + mk, so, h, :],
                                    et[:mk, :nq],
                                    start=(ki == 0), stop=(ki == len(kch) - 1))
                            rec = rcp.tile([1, 512], F32, tag="rec")
                            nc.vector.reciprocal(rec[:1, :nq], po[D:D + 1, :nq])
                            pb = apb.tile([D, 512], F32, tag="pbn")
                            nc.tensor.matmul(pb[:, :nq], ones_r[:1, :D],
                                             rec[:1, :nq])
                            sb = rcp.tile([D, 512], F32, tag="sbn")
                            nc.vector.tensor_copy(sb[:, :nq], pb[:, :nq])
                            nc.vector.tensor_mul(outT[hsl, q0:q1],
                                                 po[:D, :nq], sb[:, :nq])
                            q0 = q1

                # ---------------- phase 3: A2A + proj ----------------
                a2a_in = dram.tile([N_CORES, 128, SLC], F32)
                a2a_out = dram.tile([N_CORES, 128, SLC], F32)
                for j in range(N_CORES):
                    nc.sync.dma_start(a2a_in[j], outT[:, j * SLC:(j + 1) * SLC])
                nc.gpsimd.collective_compute(
                    "AllToAll", mybir.AluOpType.bypass,
                    replica_groups=[list(range(N_CORES))],
                    ins=[a2a_in.opt()], outs=[a2a_out.opt()])
                aT = prp.tile([128, 8, SLC], F32)
                for kc in range(N_CORES):
                    nc.sync.dma_start(aT[:, kc, :], a2a_out[kc])
                out_v = out.ap().rearrange("(sc p) e -> p sc e", p=128)
                ob = prp.tile([128, 2, E], F32)
                for sc2 in range(SLC // 128):
                    ssl = slice(sc2 * 128, (sc2 + 1) * 128)
                    for eh in range(2):
                        esl = slice(eh * 512, (eh + 1) * 512)
                        pp2 = apo.tile([128, 512], F32, tag="po")
                        for kc in range(N_CORES):
                            nc.tensor.matmul(pp2[:], aT[:, kc, ssl],
                                             proj_s[:, kc, esl],
                                             start=(kc == 0), stop=False)
                        nc.tensor.matmul(pp2[:], ones_r[:1, :], bo_s[:, esl],
                                         start=False, stop=True)
                        nc.scalar.activation(ob[:, sc2, esl], pp2[:], AF.Identity)
                    nc.sync.dma_start(out_v[:, sc2, :], ob[:, sc2, :])
    _split_multiwaits(nc)
    return nc


_CACHE = {}
LAST_RESULTS = None


def kernel(hidden_states, rotary_pos_emb, qkv_w, qkv_b, q_norm_w, k_norm_w,
           proj_w, proj_b, cu_seqlens):
    global LAST_RESULTS
    hidden_states = np.asarray(hidden_states, dtype=np.float32)
    rotary_pos_emb = np.asarray(rotary_pos_emb, dtype=np.float32)
    qkv_w = np.asarray(qkv_w, dtype=np.float32)
    qkv_b = np.asarray(qkv_b, dtype=np.float32)
    q_norm_w = np.asarray(q_norm_w, dtype=np.float32)
    k_norm_w = np.asarray(k_norm_w, dtype=np.float32)
    proj_w = np.asarray(proj_w, dtype=np.float32)
    proj_b = np.asarray(proj_b, dtype=np.float32)
    cu = np.asarray(cu_seqlens).astype(np.int64)

    key = tuple(cu.tolist())
    if key not in _CACHE:
        _CACHE[key] = _build(cu)
    nc = _CACHE[key]

    hT = np.ascontiguousarray(hidden_states.T)
    frT = np.ascontiguousarray(rotary_pos_emb.T)
    projT = np.ascontiguousarray(proj_w.T)
    bo = np.ascontiguousarray(proj_b[None, :])
    in_maps = []
    for c in range(N_CORES):
        fsl = slice(c * FPC, (c + 1) * FPC)
        in_maps.append({
            "hT": hT,
            "wqT": np.ascontiguousarray(qkv_w[fsl, :].T),
            "wkT": np.ascontiguousarray(qkv_w[E + c * FPC:E + (c + 1) * FPC, :].T),
            "wvT": np.ascontiguousarray(qkv_w[2 * E + c * FPC:2 * E + (c + 1) * FPC, :].T),
            "bq": np.ascontiguousarray(qkv_b[c * FPC:(c + 1) * FPC, None]),
            "bk": np.ascontiguousarray(qkv_b[E + c * FPC:E + (c + 1) * FPC, None]),
            "bv": np.ascontiguousarray(qkv_b[None, 2 * E + c * FPC:2 * E + (c + 1) * FPC]),
            "wqn": np.ascontiguousarray(q_norm_w[fsl, None]),
            "wkn": np.ascontiguousarray(k_norm_w[fsl, None]),
            "projT": projT,
            "bo": bo,
            "frT": frT,
        })
    res = run_bass_kernel_spmd(nc, in_maps, list(range(N_CORES)))
    LAST_RESULTS = res
    return np.concatenate([res.results[c]["out"] for c in range(N_CORES)],
                          axis=0).astype(np.float32)



# revision 3
# speedup vs baseline: 14.9881x; 14.9881x over previous
"""InternVisionAttention TRN2 kernel: 8-core tensor-parallel over heads.

Transfer-optimized revision. The axon tunnel moves ~30MB/s, so the warm
dispatch is dominated by host<->device bytes, not device compute. Changes
vs the baseline:
  - hidden_states + rotary are uploaded sequence-sharded in fp16 (540KB per
    core instead of an 8MB replicated f32 hT) and AllGathered on device.
  - proj is true row-parallel: each core holds only its [128, E] slice of
    projT; partial outputs are summed with an on-device ReduceScatter
    (replaces AllToAll + 4MB-per-core replicated projT).
  - output is written fp16 (halves the download), upcast client-side.
  - dispatch layer caches the jitted shard_map callable and keeps weight
    uploads device-resident keyed by a content digest; donated zero output
    buffers are created on device instead of uploaded.

Compute layout per core c (heads 2c..2c+1) is unchanged from the baseline:
qT/kT transposed [feat(128) x S], RMS-norm via cross-core sumsq AllReduce,
rope via partition-shifted DVE ops, per-segment attention with the
ones-column softmax-denominator trick.
"""
import hashlib
import math
import numpy as np

import jax
import jax.numpy as jnp
from jax.sharding import Mesh, NamedSharding, PartitionSpec
from jax.experimental.shard_map import shard_map

import bass_rust
import concourse.bass as bass
import concourse.mybir as mybir
import concourse.tile as tile
from concourse import bass2jax as _b2j
from concourse.vector_clock import ScopedClock

F32 = mybir.dt.float32
F16 = mybir.dt.float16
AF = mybir.ActivationFunctionType
N_CORES = 8
S, E, H, D = 2048, 1024, 16, 64
HPC = H // N_CORES          # heads per core = 2
FPC = HPC * D               # features per core = 128
SLC = S // N_CORES          # sequence slice per core = 256
PACK = E + D // 2           # packed h+rotary rows = 1056
EPS = 1e-6

# ---- walrus workaround: sync engine allows 1 sem wait per instruction ----
def _drain_and_barrier(self, tick_clock, wait_clock):
    nc = self.nc
    drain_inst = nc.sync.drain()
    wait_clock.add_sem_waits(drain_inst.ins,
                             ScopedClock({None: tick_clock.global_clock}))
    si = drain_inst.ins.sync_info
    if si is not None and len(si.on_wait) > 1:
        waits = list(si.on_wait)
        drain_inst.ins.sync_info = bass_rust.SyncInfo(
            on_wait=waits[:1], on_update=list(si.on_update))
        for i in range(1, len(waits)):
            nop = nc.sync.nop(nofuse=True)
            nop.ins.sync_info = bass_rust.SyncInfo(
                on_wait=waits[i:i + 1], on_update=[])
    nc.all_engine_barrier()
    assert self.sems is not None
    popped = nc._tile_sem_poison_stack.pop()
    assert popped is self._sem_poison
    nc.clear_and_free_semaphores(list(self.sems.allocated().values()))
    nc.all_engine_barrier()

tile.TileContext._drain_and_barrier = _drain_and_barrier


def _split_multiwaits(nc):
    """Walrus here allows only one sync wait per instruction: hoist extra
    waits onto same-engine nops inserted just before (in-order engines)."""
    n = 0
    for bb in nc.m.functions[0].blocks:
        insts = bb.instructions
        i = 0
        while i < len(insts):
            inst = insts[i]
            si = inst.sync_info
            if si is not None and len(si.on_wait) > 1:
                waits = list(si.on_wait)
                inst.sync_info = bass_rust.SyncInfo(
                    on_wait=waits[-1:], on_update=list(si.on_update))
                for w in waits[:-1]:
                    nop = mybir.InstNoOp(name=f"mwsplit_{n}",
                                         engine=inst.engine, bass_nofuse=True)
                    nop.sync_info = bass_rust.SyncInfo(on_wait=[w], on_update=[])
                    insts.insert(i, nop)
                    i += 1
                    n += 1
            i += 1


def _build(cu):
    """Build the Bass program, specialized on cu_seqlens values."""
    segs = [(int(cu[i]), int(cu[i + 1])) for i in range(len(cu) - 1)
            if int(cu[i + 1]) > int(cu[i])]

    nc = bass.Bass(num_devices=N_CORES)
    hfr = nc.dram_tensor("hfr", [PACK, SLC], F16, kind="ExternalInput")
    wqT = nc.dram_tensor("wqT", [E, FPC], F32, kind="ExternalInput")
    wkT = nc.dram_tensor("wkT", [E, FPC], F32, kind="ExternalInput")
    wvT = nc.dram_tensor("wvT", [E, FPC], F32, kind="ExternalInput")
    bq = nc.dram_tensor("bq", [FPC, 1], F32, kind="ExternalInput")
    bk = nc.dram_tensor("bk", [FPC, 1], F32, kind="ExternalInput")
    bv = nc.dram_tensor("bv", [1, FPC], F32, kind="ExternalInput")
    wqn = nc.dram_tensor("wqn", [FPC, 1], F32, kind="ExternalInput")
    wkn = nc.dram_tensor("wkn", [FPC, 1], F32, kind="ExternalInput")
    projP = nc.dram_tensor("projP", [FPC, E], F32, kind="ExternalInput")
    bo8 = nc.dram_tensor("bo8", [1, E], F32, kind="ExternalInput")
    out = nc.dram_tensor("out", [SLC, E], F16, kind="ExternalOutput")

    groups = [list(range(N_CORES))]

    with tile.TileContext(nc) as tc:
        with tc.tile_pool(name="persist", bufs=1) as pp, \
             tc.tile_pool(name="dram", bufs=1, space="DRAM") as dram:
            # persistent tiles
            wq_s = pp.tile([128, 8, FPC], F32)
            wk_s = pp.tile([128, 8, FPC], F32)
            wv_s = pp.tile([128, 8, FPC], F32)
            nc.sync.dma_start(wq_s[:], wqT.ap().rearrange("(eo p) o -> p eo o", p=128))
            nc.sync.dma_start(wk_s[:], wkT.ap().rearrange("(eo p) o -> p eo o", p=128))
            nc.sync.dma_start(wv_s[:], wvT.ap().rearrange("(eo p) o -> p eo o", p=128))
            bq_s = pp.tile([FPC, 1], F32)
            bk_s = pp.tile([FPC, 1], F32)
            bv_s = pp.tile([1, FPC], F32)
            wqn_s = pp.tile([FPC, 1], F32)
            wkn_s = pp.tile([FPC, 1], F32)
            bo8_s = pp.tile([1, E], F32)
            projP_s = pp.tile([128, E], F32)
            nc.sync.dma_start(bq_s[:], bq.ap())
            nc.sync.dma_start(bk_s[:], bk.ap())
            nc.sync.dma_start(bv_s[:], bv.ap())
            nc.sync.dma_start(wqn_s[:], wqn.ap())
            nc.sync.dma_start(wkn_s[:], wkn.ap())
            nc.sync.dma_start(bo8_s[:], bo8.ap())
            nc.sync.dma_start(projP_s[:], projP.ap())
            ones_r = pp.tile([1, 128], F32)      # ones row (K=1 lhsT tricks)
            ones_c = pp.tile([128, 1], F32)      # ones column (sumsq rhs)
            nc.vector.memset(ones_r[:], 1.0)
            nc.vector.memset(ones_c[:], 1.0)
            halfpi = pp.tile([128, 1], F32)
            nc.vector.memset(halfpi[:], math.pi / 2)
            epsq = pp.tile([1, 1], F32)
            nc.vector.memset(epsq[:], float(D) * EPS)
            epsk = pp.tile([128, 1], F32)
            nc.vector.memset(epsk[:], EPS)

            cosT = pp.tile([128, S], F32)
            sinT = pp.tile([128, S], F32)
            qT = pp.tile([128, S], F32)          # raw then roped/normed q
            kT = pp.tile([128, S], F32)
            v_s = pp.tile([128, 16, HPC, D + 1], F32)   # +ones column
            nc.vector.memset(v_s[:, :, :, D:D + 1], 1.0)
            outT = pp.tile([128, S], F32)
            sq_q = pp.tile([2, S], F32)          # row0: q sumsq, row1 unused
            ks_p = pp.tile([128, 16], F32)       # k sumsq partition-major
            fq = pp.tile([1, S], F32)
            fk = pp.tile([128, 16], F32)

            # ------------- phase 0: AllGather h+rotary (fp16) -------------
            # collectives cannot read IO tensors: bounce through internal DRAM
            hfr_i = dram.tile([PACK, SLC], F16)
            nc.sync.dma_start(hfr_i[:, :], hfr.ap())
            ag = dram.tile([N_CORES, PACK, SLC], F16)
            nc.gpsimd.collective_compute(
                "AllGather", mybir.AluOpType.bypass,
                replica_groups=groups,
                ins=[hfr_i.opt()], outs=[ag.opt()])

            # ---------------- phase 1: qkv ----------------
            with tc.tile_pool(name="hpool", bufs=1) as hp, \
                 tc.tile_pool(name="h16p", bufs=2) as h16p, \
                 tc.tile_pool(name="p1ps", bufs=2, space="PSUM") as p1ps, \
                 tc.tile_pool(name="p1pv", bufs=2, space="PSUM") as p1pv, \
                 tc.tile_pool(name="p1sq", bufs=1, space="PSUM") as p1sq, \
                 tc.tile_pool(name="sqtmp", bufs=2) as sqt:
                h_s = hp.tile([128, 8, S], F32)
                fr16 = hp.tile([128, S], F16)
                for j in range(N_CORES):
                    jsl = slice(j * SLC, (j + 1) * SLC)
                    h16 = h16p.tile([128, 8, SLC], F16, tag="h16")
                    nc.sync.dma_start(
                        h16[:], ag[j, 0:E, :].rearrange("(eo p) t -> p eo t", p=128))
                    for eo in range(8):
                        nc.scalar.activation(h_s[:, eo, jsl], h16[:, eo, :],
                                             AF.Identity)
                    for b in range(4):
                        nc.sync.dma_start(fr16[b * 32:(b + 1) * 32, jsl],
                                          ag[j, E:PACK, :])
                fr = hp.tile([128, S], F32)
                nc.scalar.activation(fr[:], fr16[:], AF.Identity)
                nc.scalar.activation(sinT[:], fr[:], AF.Sin)
                nc.scalar.activation(cosT[:], fr[:], AF.Sin, bias=halfpi[:])

                for sc in range(4):
                    sl = slice(sc * 512, (sc + 1) * 512)
                    pq = p1ps.tile([128, 512], F32, tag="pqk")
                    pk = p1ps.tile([128, 512], F32, tag="pqk")
                    for eo in range(8):
                        nc.tensor.matmul(pq[:], wq_s[:, eo, :], h_s[:, eo, sl],
                                         start=(eo == 0), stop=(eo == 7))
                    for eo in range(8):
                        nc.tensor.matmul(pk[:], wk_s[:, eo, :], h_s[:, eo, sl],
                                         start=(eo == 0), stop=(eo == 7))
                    # bias (per-partition) evac
                    nc.scalar.activation(qT[:, sl], pq[:], AF.Identity, bias=bq_s[:])
                    nc.scalar.activation(kT[:, sl], pk[:], AF.Identity, bias=bk_s[:])
                    # sumsq partials
                    qsq = sqt.tile([128, 512], F32, tag="sq")
                    ksq = sqt.tile([128, 512], F32, tag="sq")
                    nc.scalar.activation(qsq[:], qT[:, sl], AF.Square)
                    nc.scalar.activation(ksq[:], kT[:, sl], AF.Square)
                    psq = p1sq.tile([1, 512], F32, tag="psq")
                    nc.tensor.matmul(psq[:], ones_c[:], qsq[:])
                    nc.scalar.activation(sq_q[0:1, sl], psq[:], AF.Identity)
                    for ss in range(4):
                        pks = p1sq.tile([128, 1], F32, tag="pks")
                        nc.tensor.matmul(pks[:], ksq[:, ss * 128:(ss + 1) * 128],
                                         ones_c[:])
                        nc.scalar.activation(
                            ks_p[:, sc * 4 + ss:sc * 4 + ss + 1], pks[:], AF.Identity)
                    # norm-weight mul (before rope)
                    nc.vector.tensor_scalar_mul(qT[:, sl], qT[:, sl], wqn_s[:])
                    nc.vector.tensor_scalar_mul(kT[:, sl], kT[:, sl], wkn_s[:])
                    # v natural with ones-trick bias
                    for ss in range(4):
                        so = sc * 4 + ss
                        pv = p1pv.tile([128, FPC], F32, tag="pv")
                        ssl = slice(so * 128, (so + 1) * 128)
                        for eo in range(8):
                            nc.tensor.matmul(pv[:], h_s[:, eo, ssl], wv_s[:, eo, :],
                                             start=(eo == 0), stop=False)
                        nc.tensor.matmul(pv[:], ones_r[:1, :], bv_s[:],
                                         start=False, stop=True)
                        for h in range(HPC):
                            nc.scalar.activation(v_s[:, so, h, 0:D],
                                                 pv[:, h * D:(h + 1) * D], AF.Identity)

                # cross-core sumsq AllReduce (packed into one buffer)
                cc_in = dram.tile([6144], F32)
                cc_out = dram.tile([6144], F32)
                nc.sync.dma_start(
                    cc_in[0:4096].rearrange("(a b) -> a b", a=2), sq_q[:])
                nc.sync.dma_start(
                    cc_in[4096:6144].rearrange("(a b) -> a b", a=128), ks_p[:])
                nc.gpsimd.collective_compute(
                    "AllReduce", mybir.AluOpType.add,
                    replica_groups=groups,
                    ins=[cc_in.opt()], outs=[cc_out.opt()])
                nc.sync.dma_start(
                    sq_q[:], cc_out[0:4096].rearrange("(a b) -> a b", a=2))
                nc.sync.dma_start(
                    ks_p[:], cc_out[4096:6144].rearrange("(a b) -> a b", a=128))
                # fq = (1/8)*rsqrt(var+eps); fk = rsqrt(var+eps)
                nc.scalar.activation(fq[:], sq_q[0:1, :], AF.Sqrt,
                                     scale=float(D) / E, bias=epsq[:])
                nc.vector.reciprocal(fq[:], fq[:])
                nc.scalar.activation(fk[:], ks_p[:], AF.Sqrt,
                                     scale=1.0 / E, bias=epsk[:])
                nc.vector.reciprocal(fk[:], fk[:])

                # ---- rope (q,k) then q *= fq broadcast ----
                with tc.tile_pool(name="ropet", bufs=2) as rp, \
                     tc.tile_pool(name="bps", bufs=2, space="PSUM") as bps:
                    for t in (qT, kT):
                        tmp = rp.tile([128, S], F32, tag="ropetmp")
                        for h in range(HPC):
                            lo = h * D
                            mid = lo + D // 2
                            hi = lo + D
                            nc.vector.tensor_copy(tmp[lo:mid, :], t[mid:hi, :])
                            nc.vector.tensor_copy(tmp[mid:hi, :], t[lo:mid, :])
                        nc.vector.tensor_mul(tmp[:], tmp[:], sinT[:])
                        nc.vector.tensor_mul(t[:], t[:], cosT[:])
                        for h in range(HPC):
                            lo = h * D
                            mid = lo + D // 2
                            hi = lo + D
                            nc.vector.tensor_sub(t[lo:mid, :], t[lo:mid, :],
                                                 tmp[lo:mid, :])
                            nc.vector.tensor_add(t[mid:hi, :], t[mid:hi, :],
                                                 tmp[mid:hi, :])
                    for nqc in range(4):
                        sl = slice(nqc * 512, (nqc + 1) * 512)
                        pb = bps.tile([128, 512], F32, tag="pb")
                        nc.tensor.matmul(pb[:], ones_r[:1, :], fq[0:1, sl])
                        nc.vector.tensor_mul(qT[:, sl], qT[:, sl], pb[:])

            # ---------------- phase 2: attention ----------------
            with tc.tile_pool(name="expp", bufs=3) as ep, \
                 tc.tile_pool(name="recp", bufs=2) as rcp, \
                 tc.tile_pool(name="aps", bufs=3, space="PSUM") as aps, \
                 tc.tile_pool(name="apo", bufs=2, space="PSUM") as apo, \
                 tc.tile_pool(name="apb", bufs=2, space="PSUM") as apb:
                for h in range(HPC):
                    hsl = slice(h * D, (h + 1) * D)
                    for (s0, s1) in segs:
                        # k chunks on the 128 grid
                        kch = []
                        k0 = s0
                        while k0 < s1:
                            k1 = min(s1, (k0 // 128 + 1) * 128)
                            kch.append((k0, k1))
                            k0 = k1
                        q0 = s0
                        while q0 < s1:
                            q1 = min(s1, q0 + 512)
                            nq = q1 - q0
                            po = apo.tile([D + 1, 512], F32, tag="po")
                            for ki, (k0, k1) in enumerate(kch):
                                mk = k1 - k0
                                so, p0 = k0 // 128, k0 % 128
                                ps = aps.tile([128, 512], F32, tag="ps")
                                nc.tensor.matmul(ps[:mk, :nq], kT[hsl, k0:k1],
                                                 qT[hsl, q0:q1])
                                et = ep.tile([128, 512], F32, tag="et")
                                nc.scalar.activation(
                                    et[:mk, :nq], ps[:mk, :nq], AF.Exp,
                                    scale=fk[p0:p0 + mk, so:so + 1])
                                nc.tensor.matmul(
                                    po[:, :nq], v_s[p0:p0 + mk, so, h, :],
                                    et[:mk, :nq],
                                    start=(ki == 0), stop=(ki == len(kch) - 1))
                            rec = rcp.tile([1, 512], F32, tag="rec")
                            nc.vector.reciprocal(rec[:1, :nq], po[D:D + 1, :nq])
                            pb = apb.tile([D, 512], F32, tag="pbn")
                            nc.tensor.matmul(pb[:, :nq], ones_r[:1, :D],
                                             rec[:1, :nq])
                            sb = rcp.tile([D, 512], F32, tag="sbn")
                            nc.vector.tensor_copy(sb[:, :nq], pb[:, :nq])
                            nc.vector.tensor_mul(outT[hsl, q0:q1],
                                                 po[:D, :nq], sb[:, :nq])
                            q0 = q1

            # -------- phase 3: row-parallel proj + ReduceScatter --------
            with tc.tile_pool(name="obp", bufs=3) as obp, \
                 tc.tile_pool(name="p3ps", bufs=2, space="PSUM") as p3ps:
                part_d = dram.tile([S, E], F32)
                for sc in range(S // 128):
                    psl = slice(sc * 128, (sc + 1) * 128)
                    for eh in range(2):
                        esl = slice(eh * 512, (eh + 1) * 512)
                        pt = p3ps.tile([128, 512], F32, tag="p3")
                        nc.tensor.matmul(pt[:], outT[:, psl], projP_s[:, esl],
                                         start=True, stop=False)
                        nc.tensor.matmul(pt[:], ones_r[:1, :], bo8_s[:, esl],
                                         start=False, stop=True)
                        ob = obp.tile([128, 512], F32, tag="ob")
                        nc.scalar.activation(ob[:], pt[:], AF.Identity)
                        nc.sync.dma_start(part_d[psl, esl], ob[:])
                rs_d = dram.tile([SLC, E], F32)
                nc.gpsimd.collective_compute(
                    "ReduceScatter", mybir.AluOpType.add,
                    replica_groups=groups,
                    ins=[part_d.opt()], outs=[rs_d.opt()])
                rsb = obp.tile([128, 2, E], F32, tag="rsb")
                nc.sync.dma_start(
                    rsb[:], rs_d[:, :].rearrange("(sc p) e -> p sc e", p=128))
                o16 = obp.tile([128, 2, E], F16, tag="o16")
                for sc2 in range(2):
                    nc.scalar.activation(o16[:, sc2, :], rsb[:, sc2, :],
                                         AF.Identity)
                nc.sync.dma_start(
                    out.ap().rearrange("(sc p) e -> p sc e", p=128), o16[:])
    _split_multiwaits(nc)
    return nc


class _Dispatch:
    """Cached PJRT dispatch for one built Bass program.

    Mirrors bass2jax.run_bass_via_pjrt but (a) builds the jitted shard_map
    callable once, (b) keeps weight inputs device-resident across calls
    keyed by a content digest, (c) creates the donated zero output buffers
    on device instead of uploading them.
    """

    STREAMED = ("hfr",)

    def __init__(self, nc):
        _b2j.install_neuronx_cc_hook()
        assert nc.dbg_addr is None
        partition_name = (nc.partition_id_tensor.name
                          if nc.partition_id_tensor else None)
        in_names, out_names, out_avals = [], [], []
        for alloc in nc.m.functions[0].allocations:
            if not isinstance(alloc, mybir.MemoryLocationSet):
                continue
            assert alloc.memorylocations
            name = alloc.memorylocations[0].name
            if alloc.kind == "ExternalInput":
                if name != partition_name:
                    in_names.append(name)
            elif alloc.kind == "ExternalOutput":
                assert alloc.tensor_shape is not None and alloc.dtype is not None
                out_names.append(name)
                shape = tuple(alloc.tensor_shape)
                dtype = mybir.dt.np(alloc.dtype)
                out_avals.append(jax.core.ShapedArray(shape, dtype))
        self.param_names = list(in_names)
        self.out_names = list(out_names)
        n_params = len(in_names)
        n_outs = len(out_names)
        all_in_names = in_names + out_names
        if partition_name is not None:
            all_in_names.append(partition_name)

        def _body(*args):
            operands = list(args)
            if partition_name is not None:
                operands.append(_b2j.partition_id_tensor())
            outs = _b2j._bass_exec_p.bind(
                *operands,
                out_avals=tuple(out_avals),
                in_names=tuple(all_in_names),
                out_names=tuple(out_names),
                lowering_input_output_aliases=(),
                sim_require_finite=True,
                sim_require_nnan=True,
                nc=nc,
            )
            return tuple(outs)

        devices = jax.devices()[:N_CORES]
        assert len(devices) == N_CORES
        self.mesh = Mesh(np.asarray(devices), ("core",))
        self.sharding = NamedSharding(self.mesh, PartitionSpec("core"))
        in_specs = (PartitionSpec("core"),) * (n_params + n_outs)
        out_specs = (PartitionSpec("core"),) * n_outs
        donate = tuple(range(n_params, n_params + n_outs))
        self.sharded = jax.jit(
            shard_map(_body, mesh=self.mesh, in_specs=in_specs,
                      out_specs=out_specs, check_rep=False),
            donate_argnums=donate, keep_unused=True)
        zspecs = [((N_CORES * a.shape[0],) + tuple(a.shape[1:]), a.dtype)
                  for a in out_avals]
        self._mkzeros = jax.jit(
            lambda: tuple(jnp.zeros(s, d) for s, d in zspecs),
            out_shardings=tuple(self.sharding for _ in zspecs))
        self._weight_digest = None
        self._weight_dev = None

    def run(self, streamed, weight_digest, build_weights):
        """streamed: {name: global (8*dim0, ...) np array} uploaded every
        call. build_weights() -> same-form dict for the cached names, only
        invoked when weight_digest misses."""
        dev = {}
        for name, arr in streamed.items():
            dev[name] = jax.device_put(arr, self.sharding)
        if self._weight_digest != weight_digest:
            w = build_weights()
            self._weight_dev = {
                k: jax.device_put(v, self.sharding) for k, v in w.items()}
            self._weight_digest = weight_digest
        args = []
        for name in self.param_names:
            if name in dev:
                args.append(dev[name])
            else:
                args.append(self._weight_dev[name])
        zeros = self._mkzeros()
        outs = self.sharded(*args, *zeros)
        return {name: np.asarray(outs[i])
                for i, name in enumerate(self.out_names)}


_CACHE = {}
LAST_RESULTS = None


def kernel(hidden_states, rotary_pos_emb, qkv_w, qkv_b, q_norm_w, k_norm_w,
           proj_w, proj_b, cu_seqlens):
    hidden_states = np.asarray(hidden_states, dtype=np.float32)
    rotary_pos_emb = np.asarray(rotary_pos_emb, dtype=np.float32)
    qkv_w = np.asarray(qkv_w, dtype=np.float32)
    qkv_b = np.asarray(qkv_b, dtype=np.float32)
    q_norm_w = np.asarray(q_norm_w, dtype=np.float32)
    k_norm_w = np.asarray(k_norm_w, dtype=np.float32)
    proj_w = np.asarray(proj_w, dtype=np.float32)
    proj_b = np.asarray(proj_b, dtype=np.float32)
    cu = np.asarray(cu_seqlens).astype(np.int64)

    key = tuple(cu.tolist())
    if key not in _CACHE:
        _CACHE[key] = _Dispatch(_build(cu))
    disp = _CACHE[key]

    # streamed activations: packed [hT_slice; frT_slice] per core, fp16
    hT16 = hidden_states.T.astype(np.float16)          # [E, S]
    frT16 = rotary_pos_emb.T.astype(np.float16)        # [D//2, S]
    G = np.empty((N_CORES * PACK, SLC), np.float16)
    for c in range(N_CORES):
        ssl = slice(c * SLC, (c + 1) * SLC)
        G[c * PACK:c * PACK + E] = hT16[:, ssl]
        G[c * PACK + E:(c + 1) * PACK] = frT16[:, ssl]

    hsh = hashlib.blake2b(digest_size=16)
    for a in (qkv_w, qkv_b, q_norm_w, k_norm_w, proj_w, proj_b):
        hsh.update(np.ascontiguousarray(a).tobytes())
    digest = hsh.digest()

    def build_weights():
        w = {}
        for tag, off in (("wqT", 0), ("wkT", E), ("wvT", 2 * E)):
            wT = qkv_w[off:off + E].T                   # [E, E]
            w[tag] = np.ascontiguousarray(
                np.concatenate([wT[:, c * FPC:(c + 1) * FPC]
                                for c in range(N_CORES)], axis=0))
        w["bq"] = np.ascontiguousarray(qkv_b[0:E].reshape(N_CORES * FPC, 1))
        w["bk"] = np.ascontiguousarray(qkv_b[E:2 * E].reshape(N_CORES * FPC, 1))
        w["bv"] = np.ascontiguousarray(qkv_b[2 * E:3 * E].reshape(N_CORES, FPC))
        w["wqn"] = np.ascontiguousarray(q_norm_w.reshape(N_CORES * FPC, 1))
        w["wkn"] = np.ascontiguousarray(k_norm_w.reshape(N_CORES * FPC, 1))
        w["projP"] = np.ascontiguousarray(proj_w.T)     # [E, E] rows in core order
        w["bo8"] = np.ascontiguousarray(
            np.tile(proj_b[None, :] / N_CORES, (N_CORES, 1)))
        return w

    outs = disp.run({"hfr": G}, digest, build_weights)
    return outs["out"].astype(np.float32)
